# revision 28
# baseline (speedup 1.0000x reference)
# Trainium2 Bass kernel for nn_BasicTransformerBlock (sparse-causal attn +
# cross attn + geglu FFN + temporal attn), 8-core SPMD, single NEFF.
#
# Sharding (stages 1-3): core c owns frames {2c, 2c+1} of BOTH batches,
#   units ordered [(b0,f0),(b1,f0),(b0,f0+1),(b1,f0+1)] so the two
#   frame-parity halves are contiguous token halves.  Temporal stage:
#   core c owns (batch c//4, spatial tokens [64*(c%4), +64)) x 16
#   frames; reshard via two 8-way AllToAlls (bf16), one per frame-parity
#   half, each launched right after its FFN W2 half completes so the
#   first fully overlaps compute.
#
# Per-core layouts:
#   h (residual): token-major f32, 8 tiles [128, 1280] (bf16 after reshard).
#   nh (LN out): feature-major 3D tile [128, nkt, T] via DMA-XBAR
#        transposes (fp8e4 for attn projections, bf16 for FFN).
#   Q/K/V/O projections: fp8e4 DoubleRow (weights prescaled x64; unscale
#        folded into exp-scale / V-copy scale / residual-add).
#   Q/K: head-split fm bf16: 8 main tiles (128 ch) + 2 rem-pack tiles
#        (4 heads x 32 ch).  Scores: main MM + rem MM accumulate into ONE
#        PSUM group (mixed-base accumulation verified on HW).
#   softmax: exp on ACT straight from PSUM (scale=DH^-0.5/WS^2,
#        accum_out=rowsum) -> 1/l on DVE -> DMA-XBAR transpose of P ->
#        attn@V (fm out, head-split, stored fp8) -> O-proj (stationary =
#        activations) -> residual add (x 1/WS) on DVE.
import sys

sys.path.insert(0, '/opt/trn_rl_repo')

import numpy as np
import ml_dtypes

import concourse.bass as bass  # noqa: F401
import concourse.mybir as mybir
import concourse.tile as tile
from concourse import bacc, bass_utils

F32 = mybir.dt.float32
BF16 = mybir.dt.bfloat16
FP8 = mybir.dt.float8e4
AF = mybir.ActivationFunctionType
ALU = mybir.AluOpType
AX = mybir.AxisListType
DR = mybir.MatmulPerfMode.DoubleRow

DIM = 1280
HEADS = 8
DH = 160
CROSS = 768
FRAMES = 16
B = 2
TOK = 256
ESEQ = 77
INNER = 4 * DIM          # 5120
N_CORES = 8
T_OWN = 4 * TOK          # 1024
T_KV = 6 * TOK           # 1536 (6 kv blocks: b0f0,b0fp,b0f2c,b1f0,b1fp,b1f2c)
NT_OWN = T_OWN // 128    # 8
NKT = DIM // 128         # 10
NKTC = CROSS // 128      # 6
NM1 = INNER // 128       # 40
ISCALE = float(DH) ** -0.5
WS = 64.0                # fp8 weight prescale
ESC = ISCALE / (WS * WS)  # exp scale compensating Q,K both x WS
TENC = 320               # padded 4*ESEQ (308) for fp8 stride alignment

bf16 = ml_dtypes.bfloat16
fp8e4 = ml_dtypes.float8_e4m3
_CACHE = {}


def _cdiv(a, b):
    return (a + b - 1) // b


def build_program():
    nc = bacc.Bacc("TRN2", target_bir_lowering=False, debug=False,
                   num_devices=N_CORES)

    def din(name, shape, dt):
        return nc.dram_tensor(name, shape, dt, kind="ExternalInput").ap()

    h_in = din("h_own", [T_OWN, DIM], F32)
    h_halo = din("h_halo", [4 * TOK, DIM], BF16)    # [b0f0, b0fp, b1f0, b1fp]
    enc_in = din("enc_fm", [128, NKTC, TENC], FP8)  # feature-major, padded
    w = {}
    # head-split stationary bands [10 mb, 128 p, nkt, 128 c], fp8 x WS
    for nm, nkt in [("a1wq", NKT), ("a1wk", NKT), ("a2wq", NKT),
                    ("a2wk", NKTC), ("atwq", NKT), ("atwk", NKT)]:
        w[nm] = din(nm, [10, 128, nkt, 128], FP8)
    # moving bands [4 ch, 128 p, nkt, 320], fp8 x WS; O-proj rows
    # pre-permuted
    for nm, nkt in [("a1wv", NKT), ("a2wv", NKTC), ("atwv", NKT),
                    ("a1wo", NKT), ("a2wo", NKT), ("atwo", NKT)]:
        w[nm] = din(nm, [4, 128, nkt, 320], FP8)
    # ffn: W1 bands [40 m, 128 p, 10 kt, 2, 128]; W2 bands
    # [4 ch, 2 mh, 128 p, 20 m2, 320]  (bf16 - fp8 breaks tolerance)
    w["ffw1"] = din("ffw1", [NM1, 128, NKT, 2, 128], BF16)
    w["ffw2"] = din("ffw2", [4, 2, 128, 20, 320], BF16)
    lncst = {}
    for nm in ["n1w", "n1b", "n2w", "n2b", "n3w", "n3b", "ntw", "ntb",
               "a1bo", "a2bo", "ffb2", "atbo"]:
        lncst[nm] = din(nm + "_bc", [128, DIM], BF16)
    ffb1p = din("ffb1p", [128, NM1], F32)
    ffb1g = din("ffb1g", [128, NM1], F32)
    tmask = din("tmask", [128, 128], BF16)

    out_d = nc.dram_tensor("out", [T_OWN, DIM], BF16,
                           kind="ExternalOutput").ap()

    with tile.TileContext(nc) as tc:
        import contextlib
        with contextlib.ExitStack() as st:
            hpool = st.enter_context(tc.tile_pool(name="hpool", bufs=1))
            cpool = st.enter_context(tc.tile_pool(name="const", bufs=1))
            lncp = st.enter_context(tc.tile_pool(name="lncst", bufs=1))
            statp = st.enter_context(tc.tile_pool(name="stat", bufs=4))
            wst = st.enter_context(tc.tile_pool(name="wst", bufs=2))
            wmv = st.enter_context(tc.tile_pool(name="wmv", bufs=2))
            a2stack = contextlib.ExitStack()
            a2kvp = a2stack.enter_context(tc.tile_pool(name="a2kv", bufs=1))
            dramp = st.enter_context(tc.tile_pool(name="dram", bufs=1,
                                                  space="DRAM"))

            mask_sb = cpool.tile([128, 128], BF16, tag="tmask", name="tmask")
            nc.sync.dma_start(mask_sb[:], tmask[:])
            b1p_sb = cpool.tile([128, NM1], F32, tag="ffb1p", name="ffb1p")
            nc.sync.dma_start(b1p_sb[:], ffb1p[:])
            b1g_sb = cpool.tile([128, NM1], F32, tag="ffb1g", name="ffb1g")
            nc.sync.dma_start(b1g_sb[:], ffb1g[:])
            eps_sb = cpool.tile([128, 1], F32, tag="eps", name="eps")
            nc.vector.memset(eps_sb[:], 1e-5)

            h = []
            for t in range(NT_OWN):
                ht = hpool.tile([128, DIM], F32, tag=f"h{t}", name=f"h{t}")
                nc.sync.dma_start(ht[:], h_in[128 * t:128 * (t + 1), :])
                h.append(ht)

            # ---------------- helpers ----------------
            def load_c(name, tag):
                tl = lncp.tile([128, DIM], BF16, tag=tag, name=tag)
                nc.sync.dma_start(tl[:], lncst[name][:])
                return tl

            def layernorm_rows(src_tiles, w_b, b_b, lnscr):
                outs = []
                for x in src_tiles:
                    s1 = statp.tile([128, 1], F32, tag="s1", name="s1")
                    nc.vector.tensor_reduce(s1[:], x[:], AX.X, ALU.add)
                    sq = lnscr.tile([128, DIM], BF16, tag="sq", name="sq")
                    s2 = statp.tile([128, 1], F32, tag="s2", name="s2")
                    nc.scalar.activation(sq[:], x[:], AF.Square, accum_out=s2[:])
                    nmu = statp.tile([128, 1], F32, tag="nmu", name="nmu")
                    nc.vector.tensor_scalar_mul(nmu[:], s1[:], -1.0 / DIM)
                    mu2 = statp.tile([128, 1], F32, tag="mu2", name="mu2")
                    nc.vector.tensor_mul(mu2[:], nmu[:], nmu[:])
                    var = statp.tile([128, 1], F32, tag="var", name="var")
                    nc.vector.scalar_tensor_tensor(var[:], s2[:], 1.0 / DIM,
                                                   mu2[:], ALU.mult, ALU.subtract)
                    sd = statp.tile([128, 1], F32, tag="sd", name="sd")
                    nc.scalar.activation(sd[:], var[:], AF.Sqrt, bias=eps_sb[:])
                    rstd = statp.tile([128, 1], F32, tag="rstd", name="rstd")
                    nc.vector.reciprocal(rstd[:], sd[:])
                    tt = lnscr.tile([128, DIM], BF16, tag="lnt", name="lnt")
                    nc.vector.scalar_tensor_tensor(tt[:], x[:], nmu[:], w_b[:],
                                                   ALU.add, ALU.mult)
                    nh = lnscr.tile([128, DIM], BF16, tag="nh", name="nh")
                    nc.vector.scalar_tensor_tensor(nh[:], tt[:], rstd[:], b_b[:],
                                                   ALU.mult, ALU.add)
                    outs.append(nh)
                return outs

            def tm_to_fm(nh_tiles, fm_pool, stg_pool, tagpfx, T, dt):
                """LN rows -> feature-major 3D tile [128, NKT, T] via
                DMA-XBAR transposes (+ fp8 convert through bf16 staging)."""
                nt = len(nh_tiles)
                if dt == BF16:
                    fm = fm_pool.tile([128, NKT, T], BF16, tag=tagpfx,
                                      name=tagpfx)
                    for t in range(nt):
                        for c in range(NKT):
                            nc.sync.dma_start_transpose(
                                fm[:, c, 128 * t:128 * (t + 1)],
                                nh_tiles[t][:, 128 * c:128 * (c + 1)])
                    return fm
                fm = fm_pool.tile([128, NKT, T], FP8, tag=tagpfx, name=tagpfx)
                for r0 in range(0, nt, 4):
                    r1 = min(r0 + 4, nt)
                    stg = stg_pool.tile([128, NKT, 512], BF16, tag="stgb",
                                        name="stgb")
                    for t in range(r0, r1):
                        for c in range(NKT):
                            nc.sync.dma_start_transpose(
                                stg[:, c, 128 * (t - r0):128 * (t - r0 + 1)],
                                nh_tiles[t][:, 128 * c:128 * (c + 1)])
                    for j in range(NKT // 2):
                        nc.any.tensor_copy(
                            fm[:, 2 * j:2 * j + 2, 128 * r0:128 * r1],
                            stg[:, 2 * j:2 * j + 2, 0:128 * (r1 - r0)])
                return fm

            def w_hs_band(wt, mb, nkt):
                """Stationary band [128, nkt, 128] for m-block mb."""
                tl = wst.tile([128, nkt, 128], FP8, tag="wst", name="wst")
                nc.sync.dma_start(tl[:], wt[mb])
                return tl

            def project_headsplit(wt, in_fm, T, pool, ps_mm, tagpfx, nkt):
                """fp8 DoubleRow Q/K projection -> head-split bf16 tiles."""
                main = [pool.tile([128, T], BF16, tag=f"{tagpfx}m{i}",
                                  name=f"{tagpfx}m{i}") for i in range(8)]
                rpk = [pool.tile([128, T], BF16, tag=f"{tagpfx}r{i}",
                                 name=f"{tagpfx}r{i}") for i in range(2)]
                npair = nkt // 2
                for mb in range(10):
                    band = w_hs_band(wt, mb, nkt)
                    for ch in range(_cdiv(T, 512)):
                        c0, c1 = 512 * ch, min(512 * (ch + 1), T)
                        ps = ps_mm.tile([128, 512], F32, tag="mm", name="mm")
                        for j in range(npair):
                            nc.tensor.matmul(ps[:, 0:c1 - c0],
                                             band[:, 2 * j:2 * j + 2, :],
                                             in_fm[:, 2 * j:2 * j + 2, c0:c1],
                                             start=(j == 0), stop=(j == npair - 1),
                                             perf_mode=DR)
                        dst = main[mb] if mb < 8 else rpk[mb - 8]
                        nc.any.tensor_copy(dst[:, c0:c1], ps[:, 0:c1 - c0])
                return main, rpk

            def project_tm_out(wt, stat_fm, nkt, ps_mm, nrt, consumer):
                """fp8 DoubleRow O-proj: stationary = activations (fp8)."""
                npair = nkt // 2
                for ch in range(4):
                    c0, c1 = 320 * ch, 320 * (ch + 1)
                    bnd = wmv.tile([128, nkt, 320], FP8, tag="wmv", name="wmv")
                    nc.sync.dma_start(bnd[:], wt[ch])
                    for t in range(nrt):
                        ps = ps_mm.tile([128, 512], F32, tag="mm", name="mm")
                        for j in range(npair):
                            nc.tensor.matmul(
                                ps[:, 0:320],
                                stat_fm[:, 2 * j:2 * j + 2,
                                        128 * t:128 * (t + 1)],
                                bnd[:, 2 * j:2 * j + 2, :],
                                start=(j == 0), stop=(j == npair - 1),
                                perf_mode=DR)
                        consumer(t, c0, c1, ps[:, 0:320])

            def residual_project(bias_name, stat_fm, nkt, ps_mm, h_tiles, wt):
                bb = load_c(bias_name, "obias")
                for t in range(len(h_tiles)):
                    nc.vector.tensor_add(h_tiles[t][:], h_tiles[t][:], bb[:])

                def consume(t, c0, c1, ps):
                    # O-proj PSUM is x WS (fp8 weights prescaled)
                    nc.vector.scalar_tensor_tensor(h_tiles[t][:, c0:c1], ps,
                                                   1.0 / WS,
                                                   h_tiles[t][:, c0:c1],
                                                   ALU.mult, ALU.add)
                project_tm_out(wt, stat_fm, nkt, ps_mm, len(h_tiles), consume)

            def scores_psum(ps_mm, q_main, q_rpk, k_main, k_rpk, hd,
                            qsl, key_slices):
                """Main+rem score matmuls accumulated in ONE PSUM group."""
                g, j = hd // 4, hd % 4
                sm = ps_mm.tile([128, 512], F32, tag="mm", name="mm")
                for (kc, kn, oc) in key_slices:
                    nc.tensor.matmul(sm[:, oc:oc + kn],
                                     q_main[hd][:, qsl],
                                     k_main[hd][:, kc:kc + kn],
                                     start=True, stop=False)
                    nc.tensor.matmul(sm[:, oc:oc + kn],
                                     q_rpk[g][32 * j:32 * (j + 1), qsl],
                                     k_rpk[g][32 * j:32 * (j + 1), kc:kc + kn],
                                     start=False, stop=True,
                                     tile_position=(32 * j, 0),
                                     skip_group_check=True)
                return sm

            # =====================================================
            # Stage 0: attn2 K/V from encoder (independent of h -> fills
            # the PE while LN1 runs on DVE)
            # =====================================================
            with tc.tile_pool(name="ps_e", bufs=4, space="PSUM") as ps_e:
                enc_sb = a2kvp.tile([128, NKTC, TENC], FP8, tag="enc",
                                    name="enc")
                nc.sync.dma_start(enc_sb[:], enc_in[:])
                k2_main, k2_rpk = project_headsplit(w["a2wk"], enc_sb,
                                                    4 * ESEQ, a2kvp, ps_e,
                                                    "k2", NKTC)
                v2 = [a2kvp.tile([128, DIM], BF16, tag=f"v2{i}",
                                 name=f"v2{i}") for i in range(4)]
                for ch in range(4):
                    c0, c1 = 320 * ch, 320 * (ch + 1)
                    bnd = wmv.tile([128, NKTC, 320], FP8, tag="wmv",
                                   name="wmv")
                    nc.sync.dma_start(bnd[:], w["a2wv"][ch])
                    for fi in range(4):
                        ps = ps_e.tile([128, 512], F32, tag="mm", name="mm")
                        for j in range(NKTC // 2):
                            nc.tensor.matmul(
                                ps[0:77, 0:320],
                                enc_sb[:, 2 * j:2 * j + 2,
                                       77 * fi:77 * (fi + 1)],
                                bnd[:, 2 * j:2 * j + 2, :],
                                start=(j == 0), stop=(j == NKTC // 2 - 1),
                                perf_mode=DR)
                        nc.scalar.activation(v2[fi][0:77, c0:c1],
                                             ps[0:77, 0:320], AF.Copy,
                                             scale=1.0 / WS)

            # =====================================================
            # Stage 1: attn1  (sparse causal self-attention)
            # =====================================================
            w_b = load_c("n1w", "lnw")
            b_b = load_c("n1b", "lnb")
            with tc.tile_pool(name="a1qkv", bufs=1) as qkvp, \
                 tc.tile_pool(name="ps_mm1", bufs=5, space="PSUM") as ps_mm, \
                 tc.tile_pool(name="ps_avm1", bufs=1, space="PSUM") as ps_avm, \
                 tc.tile_pool(name="ps_avr1", bufs=1, space="PSUM") as ps_avr:

                with tc.tile_pool(name="a1fmo", bufs=1) as fmop:
                    with tc.tile_pool(name="a1fmh", bufs=1) as fmhp:
                        with tc.tile_pool(name="lnscr1", bufs=2) as lnscr, \
                             tc.tile_pool(name="stg1", bufs=1) as stgp, \
                             tc.tile_pool(name="halo", bufs=1) as halop:
                            halo = []
                            for t in range(8):
                                tl = halop.tile([128, DIM], BF16, tag="halo",
                                                name="halo")
                                nc.sync.dma_start(tl[:],
                                                  h_halo[128 * t:128 * (t + 1), :])
                                halo.append(tl)
                            nh_tm = layernorm_rows(h, w_b, b_b, lnscr)
                            nh_fm = tm_to_fm(nh_tm, fmop, stgp, "nhfm",
                                             T_OWN, FP8)
                            nhh_tm = layernorm_rows(halo, w_b, b_b, lnscr)
                            nhh_fm = tm_to_fm(nhh_tm, fmhp, stgp, "nhh",
                                              1024, FP8)

                        # K projection over 6 kv blocks
                        # [b0f0, b0fp, b0f2c, b1f0, b1fp, b1f2c]
                        k_main = [qkvp.tile([128, T_KV], BF16, tag=f"km{i}",
                                            name=f"km{i}") for i in range(8)]
                        k_rpk = [qkvp.tile([128, T_KV], BF16, tag=f"kr{i}",
                                           name=f"kr{i}") for i in range(2)]
                        kv_chunks = [(nhh_fm, 0, 0, 512), (nh_fm, 0, 512, 256),
                                     (nhh_fm, 512, 768, 512),
                                     (nh_fm, 256, 1280, 256)]
                        for mb in range(10):
                            band = w_hs_band(w["a1wk"], mb, NKT)
                            for (src, sc0, dc0, ncols) in kv_chunks:
                                ps = ps_mm.tile([128, 512], F32, tag="mm",
                                                name="mm")
                                for j in range(5):
                                    nc.tensor.matmul(
                                        ps[:, 0:ncols],
                                        band[:, 2 * j:2 * j + 2, :],
                                        src[:, 2 * j:2 * j + 2,
                                            sc0:sc0 + ncols],
                                        start=(j == 0), stop=(j == 4),
                                        perf_mode=DR)
                                dst = k_main[mb] if mb < 8 else k_rpk[mb - 8]
                                nc.any.tensor_copy(dst[:, dc0:dc0 + ncols],
                                                   ps[:, 0:ncols])

                        # V token-major over kv tokens: 12 tiles [128, 1280]
                        v_tm = [qkvp.tile([128, DIM], BF16, tag=f"v{i}",
                                          name=f"v{i}") for i in range(12)]
                        v_src = [(nhh_fm, 0), (nhh_fm, 128), (nhh_fm, 256),
                                 (nhh_fm, 384), (nh_fm, 0), (nh_fm, 128),
                                 (nhh_fm, 512), (nhh_fm, 640), (nhh_fm, 768),
                                 (nhh_fm, 896), (nh_fm, 256), (nh_fm, 384)]
                        for ch in range(4):
                            c0, c1 = 320 * ch, 320 * (ch + 1)
                            bnd = wmv.tile([128, NKT, 320], FP8,
                                           tag="wmv", name="wmv")
                            nc.sync.dma_start(bnd[:], w["a1wv"][ch])
                            for i, (src, sc0) in enumerate(v_src):
                                ps = ps_mm.tile([128, 512], F32, tag="mm",
                                                name="mm")
                                for j in range(5):
                                    nc.tensor.matmul(
                                        ps[:, 0:320],
                                        src[:, 2 * j:2 * j + 2,
                                            sc0:sc0 + 128],
                                        bnd[:, 2 * j:2 * j + 2, :],
                                        start=(j == 0), stop=(j == 4),
                                        perf_mode=DR)
                                nc.scalar.activation(v_tm[i][:, c0:c1],
                                                     ps[:, 0:320], AF.Copy,
                                                     scale=1.0 / WS)
                    # halo fm closed; Q projection (own tokens only)
                    q_main, q_rpk = project_headsplit(w["a1wq"], nh_fm, T_OWN,
                                                      qkvp, ps_mm, "q", NKT)

                # fm closed; attention core
                with tc.tile_pool(name="a1ao", bufs=1) as aop:
                    ao_fm = aop.tile([128, NKT, T_OWN], FP8, tag="aofm",
                                     name="aofm")
                    KB0 = [0, 3, 0, 3]
                    KB1 = [1, 4, 2, 5]
                    with tc.tile_pool(name="a1p", bufs=4) as pp:
                        for fi in range(4):
                            key_slices = [(256 * KB0[fi], 256, 0),
                                          (256 * KB1[fi], 256, 256)]
                            kvt = [2 * KB0[fi], 2 * KB0[fi] + 1,
                                   2 * KB1[fi], 2 * KB1[fi] + 1]
                            av_rem_ps = {}
                            for hd in range(HEADS):
                                g, j = hd // 4, hd % 4
                                pT = pp.tile([128, 4, 256], BF16, tag="pT",
                                             name="pT")
                                for qt in range(2):
                                    q0 = 256 * fi + 128 * qt
                                    sm = scores_psum(ps_mm, q_main, q_rpk,
                                                     k_main, k_rpk, hd,
                                                     slice(q0, q0 + 128),
                                                     key_slices)
                                    p = pp.tile([128, 512], BF16, tag="p", name="p")
                                    l = statp.tile([128, 1], F32, tag="l", name="l")
                                    nc.scalar.activation(p[:], sm[:], AF.Exp,
                                                         scale=ESC, accum_out=l[:])
                                    rinv = statp.tile([128, 1], F32, tag="rinv",
                                                      name="rinv")
                                    nc.vector.reciprocal(rinv[:], l[:])
                                    nc.vector.tensor_scalar_mul(p[:], p[:], rinv[:])
                                    for ki in range(4):
                                        nc.sync.dma_start_transpose(
                                            pT[:, ki, 128 * qt:128 * (qt + 1)],
                                            p[:, 128 * ki:128 * (ki + 1)])
                                avp = ps_avm.tile([128, 256], F32, tag="avm",
                                                  name="avm")
                                for ki in range(4):
                                    nc.tensor.matmul(
                                        avp[:],
                                        v_tm[kvt[ki]][:, 160 * hd:160 * hd + 128],
                                        pT[:, ki, :],
                                        start=(ki == 0), stop=(ki == 3))
                                nc.any.tensor_copy(
                                    ao_fm[:, hd, 256 * fi:256 * (fi + 1)], avp[:])
                                if j == 0:
                                    av_rem_ps[g] = ps_avr.tile([128, 256], F32,
                                                               tag="avr", name="avr")
                                rps = av_rem_ps[g]
                                for ki in range(4):
                                    nc.tensor.matmul(
                                        rps[32 * j:32 * (j + 1), :],
                                        v_tm[kvt[ki]][:, 160 * hd + 128:160 * hd + 160],
                                        pT[:, ki, :],
                                        start=(ki == 0), stop=(ki == 3),
                                        tile_position=(0, 32 * j))
                                if j == 3:
                                    nc.any.tensor_copy(
                                        ao_fm[:, 8 + g, 256 * fi:256 * (fi + 1)],
                                        rps[:])

                    residual_project("a1bo", ao_fm, NKT, ps_mm, h, w["a1wo"])

            # =====================================================
            # Stage 2: attn2  (cross-attention to text)
            # =====================================================
            w_b = load_c("n2w", "lnw")
            b_b = load_c("n2b", "lnb")
            with tc.tile_pool(name="a2qkv", bufs=1) as qkvp, \
                 tc.tile_pool(name="a2ao", bufs=1) as aop, \
                 tc.tile_pool(name="ps_mm2", bufs=5, space="PSUM") as ps_mm, \
                 tc.tile_pool(name="ps_avm2", bufs=1, space="PSUM") as ps_avm, \
                 tc.tile_pool(name="ps_avr2", bufs=1, space="PSUM") as ps_avr:

                with tc.tile_pool(name="a2fm", bufs=1) as fmp:
                    with tc.tile_pool(name="lnscr2", bufs=3) as lnscr, \
                         tc.tile_pool(name="stg2", bufs=1) as stgp:
                        nh_tm = layernorm_rows(h, w_b, b_b, lnscr)
                        nh_fm = tm_to_fm(nh_tm, fmp, stgp, "nhfm", T_OWN, FP8)
                    q_main, q_rpk = project_headsplit(w["a2wq"], nh_fm, T_OWN,
                                                      qkvp, ps_mm, "q", NKT)

                ao_fm = aop.tile([128, NKT, T_OWN], FP8, tag="aofm",
                                 name="aofm")
                with tc.tile_pool(name="a2p", bufs=4) as pp:
                    for fi in range(4):
                        av_rem_ps = {}
                        for hd in range(HEADS):
                            g, j = hd // 4, hd % 4
                            pT = pp.tile([128, 256], BF16, tag="pT", name="pT")
                            for qt in range(2):
                                q0 = 256 * fi + 128 * qt
                                sm = scores_psum(
                                    ps_mm, q_main, q_rpk, k2_main, k2_rpk,
                                    hd, slice(q0, q0 + 128),
                                    [(77 * fi, 77, 0)])
                                p = pp.tile([128, 128], BF16, tag="p", name="p")
                                l = statp.tile([128, 1], F32, tag="l", name="l")
                                nc.scalar.activation(p[:, 0:77], sm[:, 0:77],
                                                     AF.Exp, scale=ESC,
                                                     accum_out=l[:])
                                rinv = statp.tile([128, 1], F32, tag="rinv",
                                                  name="rinv")
                                nc.vector.reciprocal(rinv[:], l[:])
                                nc.vector.tensor_scalar_mul(p[:, 0:77],
                                                            p[:, 0:77], rinv[:])
                                nc.sync.dma_start_transpose(
                                    pT[:, 128 * qt:128 * (qt + 1)],
                                    p[:, 0:128])
                            avp = ps_avm.tile([128, 256], F32, tag="avm",
                                              name="avm")
                            nc.tensor.matmul(avp[:],
                                             v2[fi][0:77, 160 * hd:160 * hd + 128],
                                             pT[0:77, :], start=True, stop=True)
                            nc.any.tensor_copy(
                                ao_fm[:, hd, 256 * fi:256 * (fi + 1)], avp[:])
                            if j == 0:
                                av_rem_ps[g] = ps_avr.tile([128, 256], F32,
                                                           tag="avr", name="avr")
                            rps = av_rem_ps[g]
                            nc.tensor.matmul(
                                rps[32 * j:32 * (j + 1), :],
                                v2[fi][0:77, 160 * hd + 128:160 * hd + 160],
                                pT[0:77, :], start=True, stop=True,
                                tile_position=(0, 32 * j))
                            if j == 3:
                                nc.any.tensor_copy(
                                    ao_fm[:, 8 + g, 256 * fi:256 * (fi + 1)],
                                    rps[:])

                residual_project("a2bo", ao_fm, NKT, ps_mm, h, w["a2wo"])
            a2stack.close()

            # =====================================================
            # Stage 3: geglu FFN (bf16) + interleaved reshard AllToAlls
            # =====================================================
            w_b = load_c("n3w", "lnw")
            b_b = load_c("n3b", "lnb")
            cins = [dramp.tile([8, 64, DIM], BF16, tag=f"cin{hh}",
                               name=f"cin{hh}") for hh in range(2)]
            couts = dramp.tile([2, 8, 64, DIM], BF16, tag="couts",
                               name="couts")
            with tc.tile_pool(name="ffp", bufs=1) as ffp, \
                 tc.tile_pool(name="gelu", bufs=1) as gelup:

                ff_all = ffp.tile([128, NM1, T_OWN], BF16, tag="ff",
                                  name="ff")
                with tc.tile_pool(name="f3fm", bufs=1) as fmp:
                    with tc.tile_pool(name="lnscr3", bufs=3) as lnscr:
                        nh_tm = layernorm_rows(h, w_b, b_b, lnscr)
                        nh_fm = tm_to_fm(nh_tm, fmp, None, "nhfm", T_OWN,
                                         BF16)

                    # W1: one band load serves both token chunks and p/g
                    with tc.tile_pool(name="ps_pg", bufs=4,
                                      space="PSUM") as ps_pg, \
                         tc.tile_pool(name="w1bp", bufs=2) as w1bp:
                        bb = load_c("ffb2", "obias")
                        for t in range(NT_OWN):
                            nc.vector.tensor_add(h[t][:], h[t][:], bb[:])

                        for m in range(NM1):
                            w1b = w1bp.tile([128, NKT, 2, 128], BF16,
                                            tag="w1b", name="w1b")
                            nc.sync.dma_start(w1b[:], w["ffw1"][m])
                            pgs = [ps_pg.tile([128, 512], F32, tag="pg",
                                              name="pg") for _ in range(4)]
                            for kt in range(NKT):
                                for pg in range(2):
                                    for tci in range(2):
                                        nc.tensor.matmul(
                                            pgs[2 * pg + tci][:],
                                            w1b[:, kt, pg, :],
                                            nh_fm[:, kt,
                                                  512 * tci:512 * (tci + 1)],
                                            start=(kt == 0),
                                            stop=(kt == NKT - 1))
                            for tci in range(2):
                                tc0 = 512 * tci
                                gp = gelup.tile([128, 512], BF16, tag="gp",
                                                name="gp")
                                nc.scalar.activation(gp[:], pgs[2 + tci][:],
                                                     AF.Gelu,
                                                     bias=b1g_sb[:, m:m + 1])
                                nc.vector.scalar_tensor_tensor(
                                    ff_all[:, m, tc0:tc0 + 512], pgs[tci][:],
                                    b1p_sb[:, m:m + 1], gp[:],
                                    ALU.add, ALU.mult)

                # W2 in two halves; AllToAll of each half right after it
                # completes so collective overlaps remaining compute
                with tc.tile_pool(name="ps_w2", bufs=4, space="PSUM") as ps_w2, \
                     tc.tile_pool(name="w2bp", bufs=2) as w2bp, \
                     tc.tile_pool(name="cstg", bufs=1) as cstg:
                    for hh in range(2):
                        for ch in range(4):
                            c0, c1 = 320 * ch, 320 * (ch + 1)
                            w2b = [w2bp.tile([128, 20, 320], BF16,
                                             tag=f"w2b{mh}", name=f"w2b{mh}")
                                   for mh in range(2)]
                            for mh in range(2):
                                nc.sync.dma_start(w2b[mh][:], w["ffw2"][ch, mh])
                            for tt in range(4 * hh, 4 * hh + 4):
                                ps = ps_w2.tile([128, 512], F32, tag="w2",
                                                name="w2")
                                for m in range(NM1):
                                    nc.tensor.matmul(
                                        ps[:, 0:320],
                                        ff_all[:, m, 128 * tt:128 * (tt + 1)],
                                        w2b[m // 20][:, m % 20, :],
                                        start=(m == 0), stop=(m == NM1 - 1))
                                nc.vector.tensor_add(h[tt][:, c0:c1],
                                                     h[tt][:, c0:c1],
                                                     ps[:, 0:320])
                        # this half of h is final: stage + send its AllToAll
                        csts = []
                        for q in range(4):
                            cst = cstg.tile([128, DIM], BF16, tag=f"cs{q}",
                                            name=f"cs{q}")
                            nc.any.tensor_copy(cst[:], h[4 * hh + q][:])
                            csts.append(cst)
                        for jj in range(8):
                            u = 2 * hh + jj // 4
                            r0 = 256 * u + 64 * (jj % 4)
                            lt = r0 // 128 - 4 * hh
                            ro = r0 % 128
                            nc.sync.dma_start(cins[hh][jj],
                                              csts[lt][ro:ro + 64, :])
                        nc.gpsimd.collective_compute(
                            "AllToAll", ALU.bypass,
                            replica_groups=[[0, 1, 2, 3, 4, 5, 6, 7]],
                            ins=[cins[hh].opt()], outs=[couts[hh].opt()])

            # reload resharded rows: row = dl*16 + hh*8 + pc
            # (frame = 2*pc + hh; host unpermutes)
            h4stack = contextlib.ExitStack()
            h4pool = h4stack.enter_context(tc.tile_pool(name="h4", bufs=1))
            h4 = []
            for i in range(8):
                t4 = h4pool.tile([128, DIM], BF16, tag=f"h4_{i}",
                                 name=f"h4_{i}")
                nc.sync.dma_start(
                    t4[:],
                    couts[:, :, 8 * i:8 * (i + 1), :]
                    .rearrange("h p d c -> d h p c"))
                h4.append(t4)

            # =====================================================
            # Stage 4: temporal self-attention over frames
            # =====================================================
            w_b = load_c("ntw", "lnw")
            b_b = load_c("ntb", "lnb")
            with tc.tile_pool(name="tqkv", bufs=1) as qkvp, \
                 tc.tile_pool(name="tao", bufs=1) as aop, \
                 tc.tile_pool(name="ps_mmt", bufs=5, space="PSUM") as ps_mm, \
                 tc.tile_pool(name="ps_avmt", bufs=1, space="PSUM") as ps_avm, \
                 tc.tile_pool(name="ps_avrt", bufs=1, space="PSUM") as ps_avr:

                with tc.tile_pool(name="tfm", bufs=1) as fmp:
                    with tc.tile_pool(name="lnscrt", bufs=3) as lnscr, \
                         tc.tile_pool(name="stgt", bufs=1) as stgp:
                        nh_tm = layernorm_rows(h4, w_b, b_b, lnscr)
                        nh_fm = tm_to_fm(nh_tm, fmp, stgp, "nhfm", T_OWN, FP8)

                    q_main, q_rpk = project_headsplit(w["atwq"], nh_fm, T_OWN,
                                                      qkvp, ps_mm, "q", NKT)
                    k_main, k_rpk = project_headsplit(w["atwk"], nh_fm, T_OWN,
                                                      qkvp, ps_mm, "k", NKT)
                    v_tm = [qkvp.tile([128, DIM], BF16, tag=f"v{i}",
                                      name=f"v{i}") for i in range(8)]
                    for ch in range(4):
                        c0, c1 = 320 * ch, 320 * (ch + 1)
                        bnd = wmv.tile([128, NKT, 320], FP8, tag="wmv",
                                       name="wmv")
                        nc.sync.dma_start(bnd[:], w["atwv"][ch])
                        for t in range(8):
                            ps = ps_mm.tile([128, 512], F32, tag="mm",
                                            name="mm")
                            for j in range(5):
                                nc.tensor.matmul(
                                    ps[:, 0:320],
                                    nh_fm[:, 2 * j:2 * j + 2,
                                          128 * t:128 * (t + 1)],
                                    bnd[:, 2 * j:2 * j + 2, :],
                                    start=(j == 0), stop=(j == 4),
                                    perf_mode=DR)
                            nc.scalar.activation(v_tm[t][:, c0:c1],
                                                 ps[:, 0:320], AF.Copy,
                                                 scale=1.0 / WS)

                ao_fm = aop.tile([128, NKT, T_OWN], FP8, tag="aofm",
                                 name="aofm")
                with tc.tile_pool(name="tp", bufs=4) as pp:
                    for gdx in range(8):
                        g0 = 128 * gdx
                        av_rem_ps = {}
                        for hd in range(HEADS):
                            g, j = hd // 4, hd % 4
                            sm = scores_psum(ps_mm, q_main, q_rpk,
                                             k_main, k_rpk, hd,
                                             slice(g0, g0 + 128),
                                             [(g0, 128, 0)])
                            p = pp.tile([128, 128], BF16, tag="p", name="p")
                            nc.scalar.activation(p[:], sm[:, 0:128], AF.Exp,
                                                 scale=ESC)
                            l = statp.tile([128, 1], F32, tag="l", name="l")
                            nc.vector.scalar_tensor_tensor(p[:], p[:], 1.0,
                                                           mask_sb[:], ALU.mult,
                                                           ALU.mult,
                                                           accum_out=l[:])
                            rinv = statp.tile([128, 1], F32, tag="rinv",
                                              name="rinv")
                            nc.vector.reciprocal(rinv[:], l[:])
                            nc.vector.tensor_scalar_mul(p[:], p[:], rinv[:])
                            pT = pp.tile([128, 128], BF16, tag="pT", name="pT")
                            nc.sync.dma_start_transpose(pT[:], p[:])
                            avp = ps_avm.tile([128, 128], F32, tag="avm",
                                              name="avm")
                            nc.tensor.matmul(avp[:],
                                             v_tm[gdx][:, 160 * hd:160 * hd + 128],
                                             pT[:], start=True, stop=True)
                            nc.any.tensor_copy(ao_fm[:, hd, g0:g0 + 128],
                                               avp[:])
                            if j == 0:
                                av_rem_ps[g] = ps_avr.tile([128, 128], F32,
                                                           tag="avr", name="avr")
                            rps = av_rem_ps[g]
                            nc.tensor.matmul(
                                rps[32 * j:32 * (j + 1), :],
                                v_tm[gdx][:, 160 * hd + 128:160 * hd + 160],
                                pT[:], start=True, stop=True,
                                tile_position=(0, 32 * j))
                            if j == 3:
                                nc.any.tensor_copy(ao_fm[:, 8 + g, g0:g0 + 128],
                                                   rps[:])

                residual_project("atbo", ao_fm, NKT, ps_mm, h4, w["atwo"])

            for t in range(NT_OWN):
                nc.sync.dma_start(out_d[128 * t:128 * (t + 1), :], h4[t][:])
            h4stack.close()

    nc.compile()
    return nc


# ================= host side =================

def _prep_inputs(inputs):
    hs = np.ascontiguousarray(np.asarray(inputs["hidden_states"], np.float32))
    enc = np.ascontiguousarray(np.asarray(inputs["encoder_hidden_states"],
                                          np.float32))
    vl = int(np.asarray(inputs["video_length"]))
    assert vl == FRAMES and hs.shape == (B * FRAMES, TOK, DIM)

    def _q8(x):
        return np.clip(x, -240.0, 240.0).astype(fp8e4)

    def _hs_tiles(wt):
        """[Kin, 1280] -> [10 mb, 128 p, nkt, 128 c] head-split bands."""
        kin = wt.shape[0]
        nkt = kin // 128
        out = np.empty((10, 128, nkt, 128), np.float32)
        w3 = wt.reshape(nkt, 128, HEADS, DH)   # [kt, p, h, c]
        for mb in range(8):
            out[mb] = w3[:, :, mb, 0:128].transpose(1, 0, 2)
        for g in range(2):
            rem = w3[:, :, 4 * g:4 * (g + 1), 128:160]  # [kt, p, 4, 32]
            out[8 + g] = rem.reshape(nkt, 128, 128).transpose(1, 0, 2)
        return out

    def _mv_tiles(wt):
        """[Kin, 1280] -> [4 ch, 128 p, nkt, 320] moving bands."""
        kin = wt.shape[0]
        nkt = kin // 128
        return np.ascontiguousarray(
            wt.reshape(nkt, 128, 4, 320).transpose(2, 1, 0, 3))

    def _wo_perm(wt):
        """Permute O-proj rows into head-split order, then moving bands."""
        w3 = wt.reshape(HEADS, DH, DIM)
        rows = [w3[hd, 0:128] for hd in range(8)]
        rows += [w3[4 * g:4 * (g + 1), 128:160].reshape(128, DIM)
                 for g in range(2)]
        return _mv_tiles(np.concatenate(rows, 0))

    gw = lambda k: np.asarray(inputs[k], np.float32)
    ffw1 = gw("ffw1")
    ffw1_t = np.empty((NM1, 128, NKT, 2, 128), np.float32)
    for m in range(NM1):
        for kt in range(NKT):
            ks = slice(128 * kt, 128 * (kt + 1))
            ffw1_t[m, :, kt, 0, :] = ffw1[ks, 128 * m:128 * (m + 1)]
            ffw1_t[m, :, kt, 1, :] = ffw1[ks,
                                          INNER + 128 * m:INNER + 128 * (m + 1)]
    # W2 [5120, 1280] -> [4 ch, 2 mh, 128 p, 20 m2, 320]
    ffw2_t = np.ascontiguousarray(
        gw("ffw2").reshape(2, 20, 128, 4, 320).transpose(3, 0, 2, 1, 4))

    # fp8 weights, prescaled x WS
    wb8 = {
        "a1wq": _hs_tiles(gw("a1wq")), "a1wk": _hs_tiles(gw("a1wk")),
        "a2wq": _hs_tiles(gw("a2wq")), "a2wk": _hs_tiles(gw("a2wk")),
        "atwq": _hs_tiles(gw("atwq")), "atwk": _hs_tiles(gw("atwk")),
        "a1wv": _mv_tiles(gw("a1wv")), "a2wv": _mv_tiles(gw("a2wv")),
        "atwv": _mv_tiles(gw("atwv")),
        "a1wo": _wo_perm(gw("a1wo")), "a2wo": _wo_perm(gw("a2wo")),
        "atwo": _wo_perm(gw("atwo")),
    }
    wb8 = {k: np.ascontiguousarray(_q8(v * WS)) for k, v in wb8.items()}
    wb = {"ffw1": np.ascontiguousarray(ffw1_t.astype(bf16)),
          "ffw2": np.ascontiguousarray(ffw2_t.astype(bf16))}
    bc = {}
    for k in ["n1w", "n1b", "n2w", "n2b", "n3w", "n3b", "ntw", "ntb",
              "a1bo", "a2bo", "ffb2", "atbo"]:
        v = np.asarray(inputs[k], np.float32)
        bc[k + "_bc"] = np.ascontiguousarray(
            np.broadcast_to(v[None, :], (128, DIM)).astype(bf16))
    ffb1 = np.asarray(inputs["ffb1"], np.float32)
    ffb1p = np.ascontiguousarray(ffb1[:INNER].reshape(NM1, 128).T)
    ffb1g = np.ascontiguousarray(ffb1[INNER:].reshape(NM1, 128).T)
    tmask = np.ascontiguousarray(
        np.kron(np.eye(8, dtype=np.float32),
                np.ones((16, 16), np.float32)).astype(bf16))

    in_maps = []
    for c in range(N_CORES):
        f0 = 2 * c
        fp = max(f0 - 1, 0)
        units = [(0, f0), (1, f0), (0, f0 + 1), (1, f0 + 1)]
        h_own = np.concatenate([hs[b * FRAMES + f] for (b, f) in units], 0)
        h_halo = np.concatenate([hs[0], hs[fp], hs[FRAMES],
                                 hs[FRAMES + fp]], 0).astype(bf16)
        enc_c = np.concatenate([enc[b * FRAMES + f] for (b, f) in units], 0)
        # feature-major fp8, padded to TENC cols: [128, NKTC, TENC]
        enc_fm = np.zeros((128, NKTC, TENC), np.float32)
        ka = enc_c.T.reshape(NKTC, 128, 4 * ESEQ).transpose(1, 0, 2)
        enc_fm[:, :, :4 * ESEQ] = ka
        m = {"h_own": np.ascontiguousarray(h_own),
             "h_halo": np.ascontiguousarray(h_halo),
             "enc_fm": np.ascontiguousarray(_q8(enc_fm)),
             "ffb1p": ffb1p, "ffb1g": ffb1g, "tmask": tmask}
        m.update(wb8)
        m.update(wb)
        m.update(bc)
        in_maps.append(m)
    return in_maps


# row position of frame f within a 16-row dl group after the reshard
_POSOF = [(f % 2) * 8 + f // 2 for f in range(FRAMES)]


def _assemble(results):
    full = np.empty((B, FRAMES, TOK, DIM), np.float32)
    for c in range(N_CORES):
        o = np.asarray(results[c]["out"], np.float32).reshape(64, FRAMES, DIM)
        b, d0 = c // 4, 64 * (c % 4)
        full[b, :, d0:d0 + 64, :] = o[:, _POSOF, :].transpose(1, 0, 2)
    return full.reshape(B * FRAMES, TOK, DIM)


def _get_nc():
    if "nc" not in _CACHE:
        _CACHE["nc"] = build_program()
    return _CACHE["nc"]


def kernel(**inputs):
    nc = _get_nc()
    in_maps = _prep_inputs(inputs)
    res = bass_utils.run_bass_kernel_spmd(nc, in_maps,
                                          core_ids=list(range(N_CORES)))
    return _assemble(res.results)


# revision 38
# speedup vs baseline: 1.3741x; 1.3741x over previous
# Trainium2 Bass kernel for nn_BasicTransformerBlock (sparse-causal attn +
# cross attn + geglu FFN + temporal attn), 8-core SPMD, single NEFF.
#
# Sharding (stages 1-3): core c owns frames {2c, 2c+1} of BOTH batches,
#   units ordered [(b0,f0),(b1,f0),(b0,f0+1),(b1,f0+1)] so the two
#   frame-parity halves are contiguous token halves.  Temporal stage:
#   core c owns (batch c//4, spatial tokens [64*(c%4), +64)) x 16
#   frames; reshard via two 8-way AllToAlls (bf16), one per frame-parity
#   half, each launched right after its FFN W2 half completes so the
#   first fully overlaps compute.
#
# Per-core layouts:
#   h (residual): token-major f32, 8 tiles [128, 1280] (bf16 after reshard).
#   nh (LN out): LN via bn_stats/bn_aggr; feature-major 3D tile
#        [128, nkt, T] via PE transposes (fp8e4 for attn projections,
#        bf16 for FFN).
#   Q/K/V/O projections: fp8e4 DoubleRow (weights prescaled x64; unscale
#        folded into exp-scale / V-copy scale / residual-add).  attn2 K/V
#        (encoder-only) run at program start to fill the PE during LN1.
#   Q/K: head-split fm bf16: 8 main tiles (128 ch) + 2 rem-pack tiles
#        (4 heads x 32 ch).  Scores: main MM + rem MM accumulate into ONE
#        PSUM group (mixed-base accumulation verified on HW).
#   softmax: exp on ACT straight from PSUM (scale=DH^-0.5/WS^2,
#        accum_out=rowsum) -> 1/l on DVE -> PE transpose of P ->
#        attn@V (fm out, head-split, stored fp8) -> O-proj (stationary =
#        activations) -> residual add (x 1/WS) on DVE.
import sys

sys.path.insert(0, '/opt/trn_rl_repo')

import numpy as np
import ml_dtypes

import concourse.bass as bass  # noqa: F401
import concourse.mybir as mybir
import concourse.tile as tile
from concourse import bacc, bass_utils
from concourse.masks import make_identity

F32 = mybir.dt.float32
BF16 = mybir.dt.bfloat16
FP8 = mybir.dt.float8e4
AF = mybir.ActivationFunctionType
ALU = mybir.AluOpType
AX = mybir.AxisListType
DR = mybir.MatmulPerfMode.DoubleRow

DIM = 1280
HEADS = 8
DH = 160
CROSS = 768
FRAMES = 16
B = 2
TOK = 256
ESEQ = 77
INNER = 4 * DIM          # 5120
N_CORES = 8
T_OWN = 4 * TOK          # 1024
T_KV = 6 * TOK           # 1536 (6 kv blocks: b0f0,b0fp,b0f2c,b1f0,b1fp,b1f2c)
NT_OWN = T_OWN // 128    # 8
NKT = DIM // 128         # 10
NKTC = CROSS // 128      # 6
NM1 = INNER // 128       # 40
ISCALE = float(DH) ** -0.5
WS = 64.0                # fp8 weight prescale
ESC = ISCALE / (WS * WS)  # exp scale compensating Q,K both x WS
TENC = 320               # padded 4*ESEQ (308) for fp8 stride alignment

bf16 = ml_dtypes.bfloat16
fp8e4 = ml_dtypes.float8_e4m3
_CACHE = {}


def _cdiv(a, b):
    return (a + b - 1) // b


def build_program():
    nc = bacc.Bacc("TRN2", target_bir_lowering=False, debug=False,
                   num_devices=N_CORES)

    def din(name, shape, dt):
        return nc.dram_tensor(name, shape, dt, kind="ExternalInput").ap()

    h_in = din("h_own", [T_OWN, DIM], F32)
    h_halo = din("h_halo", [4 * TOK, DIM], BF16)    # [b0f0, b0fp, b1f0, b1fp]
    enc_in = din("enc_fm", [128, NKTC, TENC], FP8)  # feature-major, padded
    w = {}
    # head-split stationary bands [10 mb, 128 p, nkt, 128 c], fp8 x WS
    for nm, nkt in [("a1wq", NKT), ("a1wk", NKT), ("a2wq", NKT),
                    ("a2wk", NKTC), ("atwq", NKT), ("atwk", NKT)]:
        w[nm] = din(nm, [10, 128, nkt, 128], FP8)
    # moving bands [4 ch, 128 p, nkt, 320], fp8 x WS; O-proj rows
    # pre-permuted
    for nm, nkt in [("a1wv", NKT), ("a2wv", NKTC), ("atwv", NKT),
                    ("a1wo", NKT), ("a2wo", NKT), ("atwo", NKT)]:
        w[nm] = din(nm, [4, 128, nkt, 320], FP8)
    # ffn: W1 bands [40 m, 128 p, 10 kt, 2, 128]; W2 bands
    # [4 ch, 2 mh, 128 p, 20 m2, 320]  (bf16 - fp8 breaks tolerance)
    w["ffw1"] = din("ffw1", [NM1, 128, NKT, 2, 128], BF16)
    w["ffw2"] = din("ffw2", [4, 2, 128, 20, 320], BF16)
    lncst = {}
    for nm in ["n1w", "n1b", "n2w", "n2b", "n3w", "n3b", "ntw", "ntb",
               "a1bo", "a2bo", "ffb2", "atbo"]:
        lncst[nm] = din(nm + "_bc", [128, DIM], BF16)
    ffb1p = din("ffb1p", [128, NM1], F32)
    ffb1g = din("ffb1g", [128, NM1], F32)
    tmask = din("tmask", [128, 128], BF16)

    out_d = nc.dram_tensor("out", [T_OWN, DIM], BF16,
                           kind="ExternalOutput").ap()

    with tile.TileContext(nc) as tc:
        import contextlib
        with contextlib.ExitStack() as st:
            hpool = st.enter_context(tc.tile_pool(name="hpool", bufs=1))
            cpool = st.enter_context(tc.tile_pool(name="const", bufs=1))
            lncp = st.enter_context(tc.tile_pool(name="lncst", bufs=1))
            statp = st.enter_context(tc.tile_pool(name="stat", bufs=4))
            wst = st.enter_context(tc.tile_pool(name="wst", bufs=2))
            wmv = st.enter_context(tc.tile_pool(name="wmv", bufs=2))
            a2stack = contextlib.ExitStack()
            a2kvp = a2stack.enter_context(tc.tile_pool(name="a2kv", bufs=1))
            dramp = st.enter_context(tc.tile_pool(name="dram", bufs=1,
                                                  space="DRAM"))

            ident = cpool.tile([128, 128], BF16, tag="ident", name="ident")
            make_identity(nc, ident[:])
            mask_sb = cpool.tile([128, 128], BF16, tag="tmask", name="tmask")
            nc.sync.dma_start(mask_sb[:], tmask[:])
            b1p_sb = cpool.tile([128, NM1], F32, tag="ffb1p", name="ffb1p")
            nc.sync.dma_start(b1p_sb[:], ffb1p[:])
            b1g_sb = cpool.tile([128, NM1], F32, tag="ffb1g", name="ffb1g")
            nc.sync.dma_start(b1g_sb[:], ffb1g[:])
            eps_sb = cpool.tile([128, 1], F32, tag="eps", name="eps")
            nc.vector.memset(eps_sb[:], 1e-5)

            h = []
            for t in range(NT_OWN):
                ht = hpool.tile([128, DIM], F32, tag=f"h{t}", name=f"h{t}")
                nc.sync.dma_start(ht[:], h_in[128 * t:128 * (t + 1), :])
                h.append(ht)

            # ---------------- helpers ----------------
            def load_c(name, tag):
                tl = lncp.tile([128, DIM], BF16, tag=tag, name=tag)
                nc.sync.dma_start(tl[:], lncst[name][:])
                return tl

            def layernorm_rows(src_tiles, w_b, b_b, lnscr):
                outs = []
                for x in src_tiles:
                    bst = statp.tile([128, 3, 6], F32, tag="bst", name="bst")
                    for ci, (o, n) in enumerate([(0, 512), (512, 512),
                                                 (1024, 256)]):
                        nc.vector.bn_stats(bst[:, ci, :], x[:, o:o + n])
                    mv = statp.tile([128, 2], F32, tag="mv", name="mv")
                    nc.vector.bn_aggr(mv[:], bst[:])
                    sd = statp.tile([128, 1], F32, tag="sd", name="sd")
                    nc.scalar.activation(sd[:], mv[:, 1:2], AF.Sqrt,
                                         bias=eps_sb[:])
                    rstd = statp.tile([128, 1], F32, tag="rstd", name="rstd")
                    nc.vector.reciprocal(rstd[:], sd[:])
                    tt = lnscr.tile([128, DIM], BF16, tag="lnt", name="lnt")
                    nc.vector.scalar_tensor_tensor(tt[:], x[:], mv[:, 0:1],
                                                   w_b[:],
                                                   ALU.subtract, ALU.mult)
                    nh = lnscr.tile([128, DIM], BF16, tag="nh", name="nh")
                    nc.vector.scalar_tensor_tensor(nh[:], tt[:], rstd[:], b_b[:],
                                                   ALU.mult, ALU.add)
                    outs.append(nh)
                return outs

            def tm_to_fm(nh_tiles, fm_pool, ps_tr, tagpfx, T, dt):
                """LN rows -> feature-major 3D tile [128, NKT, T] via PE
                transposes (PSUM->SBUF copy converts dtype)."""
                fm = fm_pool.tile([128, NKT, T], dt, tag=tagpfx, name=tagpfx)
                for t in range(len(nh_tiles)):
                    for c in range(NKT):
                        pst = ps_tr.tile([128, 128], BF16, tag="tr", name="tr")
                        nc.tensor.transpose(pst[:],
                                            nh_tiles[t][:, 128 * c:128 * (c + 1)],
                                            ident[:])
                        nc.any.tensor_copy(fm[:, c, 128 * t:128 * (t + 1)],
                                           pst[:])
                return fm

            def w_hs_band(wt, mb, nkt):
                """Stationary band [128, nkt, 128] for m-block mb."""
                tl = wst.tile([128, nkt, 128], FP8, tag="wst", name="wst")
                nc.sync.dma_start(tl[:], wt[mb])
                return tl

            def project_headsplit(wt, in_fm, T, pool, ps_mm, tagpfx, nkt):
                """fp8 DoubleRow Q/K projection -> head-split bf16 tiles."""
                main = [pool.tile([128, T], BF16, tag=f"{tagpfx}m{i}",
                                  name=f"{tagpfx}m{i}") for i in range(8)]
                rpk = [pool.tile([128, T], BF16, tag=f"{tagpfx}r{i}",
                                 name=f"{tagpfx}r{i}") for i in range(2)]
                npair = nkt // 2
                for mb in range(10):
                    band = w_hs_band(wt, mb, nkt)
                    for ch in range(_cdiv(T, 512)):
                        c0, c1 = 512 * ch, min(512 * (ch + 1), T)
                        ps = ps_mm.tile([128, 512], F32, tag="mm", name="mm")
                        for j in range(npair):
                            nc.tensor.matmul(ps[:, 0:c1 - c0],
                                             band[:, 2 * j:2 * j + 2, :],
                                             in_fm[:, 2 * j:2 * j + 2, c0:c1],
                                             start=(j == 0), stop=(j == npair - 1),
                                             perf_mode=DR)
                        dst = main[mb] if mb < 8 else rpk[mb - 8]
                        nc.any.tensor_copy(dst[:, c0:c1], ps[:, 0:c1 - c0])
                return main, rpk

            def project_tm_out(wt, stat_fm, nkt, ps_mm, nrt, consumer):
                """fp8 DoubleRow O-proj: stationary = activations (fp8)."""
                npair = nkt // 2
                for ch in range(4):
                    c0, c1 = 320 * ch, 320 * (ch + 1)
                    bnd = wmv.tile([128, nkt, 320], FP8, tag="wmv", name="wmv")
                    nc.sync.dma_start(bnd[:], wt[ch])
                    for t in range(nrt):
                        ps = ps_mm.tile([128, 512], F32, tag="mm", name="mm")
                        for j in range(npair):
                            nc.tensor.matmul(
                                ps[:, 0:320],
                                stat_fm[:, 2 * j:2 * j + 2,
                                        128 * t:128 * (t + 1)],
                                bnd[:, 2 * j:2 * j + 2, :],
                                start=(j == 0), stop=(j == npair - 1),
                                perf_mode=DR)
                        consumer(t, c0, c1, ps[:, 0:320])

            def residual_project(bias_name, stat_fm, nkt, ps_mm, h_tiles, wt):
                bb = load_c(bias_name, "obias")
                for t in range(len(h_tiles)):
                    nc.vector.tensor_add(h_tiles[t][:], h_tiles[t][:], bb[:])

                def consume(t, c0, c1, ps):
                    # O-proj PSUM is x WS (fp8 weights prescaled)
                    nc.vector.scalar_tensor_tensor(h_tiles[t][:, c0:c1], ps,
                                                   1.0 / WS,
                                                   h_tiles[t][:, c0:c1],
                                                   ALU.mult, ALU.add)
                project_tm_out(wt, stat_fm, nkt, ps_mm, len(h_tiles), consume)

            def scores_psum(ps_mm, q_main, q_rpk, k_main, k_rpk, hd,
                            qsl, key_slices):
                """Main+rem score matmuls accumulated in ONE PSUM group."""
                g, j = hd // 4, hd % 4
                sm = ps_mm.tile([128, 512], F32, tag="mm", name="mm")
                for (kc, kn, oc) in key_slices:
                    nc.tensor.matmul(sm[:, oc:oc + kn],
                                     q_main[hd][:, qsl],
                                     k_main[hd][:, kc:kc + kn],
                                     start=True, stop=False)
                    nc.tensor.matmul(sm[:, oc:oc + kn],
                                     q_rpk[g][32 * j:32 * (j + 1), qsl],
                                     k_rpk[g][32 * j:32 * (j + 1), kc:kc + kn],
                                     start=False, stop=True,
                                     tile_position=(32 * j, 0),
                                     skip_group_check=True)
                return sm

            # =====================================================
            # Stage 0: attn2 K/V from encoder (independent of h -> fills
            # the PE while LN1 runs on DVE)
            # =====================================================
            with tc.tile_pool(name="ps_e", bufs=4, space="PSUM") as ps_e:
                enc_sb = a2kvp.tile([128, NKTC, TENC], FP8, tag="enc",
                                    name="enc")
                nc.sync.dma_start(enc_sb[:], enc_in[:])
                k2_main, k2_rpk = project_headsplit(w["a2wk"], enc_sb,
                                                    4 * ESEQ, a2kvp, ps_e,
                                                    "k2", NKTC)
                v2 = [a2kvp.tile([128, DIM], BF16, tag=f"v2{i}",
                                 name=f"v2{i}") for i in range(4)]
                for ch in range(4):
                    c0, c1 = 320 * ch, 320 * (ch + 1)
                    bnd = wmv.tile([128, NKTC, 320], FP8, tag="wmv",
                                   name="wmv")
                    nc.sync.dma_start(bnd[:], w["a2wv"][ch])
                    for fi in range(4):
                        ps = ps_e.tile([128, 512], F32, tag="mm", name="mm")
                        for j in range(NKTC // 2):
                            nc.tensor.matmul(
                                ps[0:77, 0:320],
                                enc_sb[:, 2 * j:2 * j + 2,
                                       77 * fi:77 * (fi + 1)],
                                bnd[:, 2 * j:2 * j + 2, :],
                                start=(j == 0), stop=(j == NKTC // 2 - 1),
                                perf_mode=DR)
                        nc.scalar.activation(v2[fi][0:77, c0:c1],
                                             ps[0:77, 0:320], AF.Copy,
                                             scale=1.0 / WS)

            # =====================================================
            # Stage 1: attn1  (sparse causal self-attention)
            # =====================================================
            w_b = load_c("n1w", "lnw")
            b_b = load_c("n1b", "lnb")
            with tc.tile_pool(name="a1qkv", bufs=1) as qkvp, \
                 tc.tile_pool(name="ps_mm1", bufs=4, space="PSUM") as ps_mm, \
                 tc.tile_pool(name="ps_tr1", bufs=2, space="PSUM") as ps_tr, \
                 tc.tile_pool(name="ps_avm1", bufs=1, space="PSUM") as ps_avm, \
                 tc.tile_pool(name="ps_avr1", bufs=1, space="PSUM") as ps_avr:

                with tc.tile_pool(name="a1fmo", bufs=1) as fmop:
                    with tc.tile_pool(name="a1fmh", bufs=1) as fmhp:
                        with tc.tile_pool(name="lnscr1", bufs=2) as lnscr, \
                             tc.tile_pool(name="halo", bufs=1) as halop:
                            halo = []
                            for t in range(8):
                                tl = halop.tile([128, DIM], BF16, tag="halo",
                                                name="halo")
                                nc.sync.dma_start(tl[:],
                                                  h_halo[128 * t:128 * (t + 1), :])
                                halo.append(tl)
                            nh_tm = layernorm_rows(h, w_b, b_b, lnscr)
                            nh_fm = tm_to_fm(nh_tm, fmop, ps_tr, "nhfm",
                                             T_OWN, FP8)
                            nhh_tm = layernorm_rows(halo, w_b, b_b, lnscr)
                            nhh_fm = tm_to_fm(nhh_tm, fmhp, ps_tr, "nhh",
                                              1024, FP8)

                        # K projection over 6 kv blocks
                        # [b0f0, b0fp, b0f2c, b1f0, b1fp, b1f2c]
                        k_main = [qkvp.tile([128, T_KV], BF16, tag=f"km{i}",
                                            name=f"km{i}") for i in range(8)]
                        k_rpk = [qkvp.tile([128, T_KV], BF16, tag=f"kr{i}",
                                           name=f"kr{i}") for i in range(2)]
                        kv_chunks = [(nhh_fm, 0, 0, 512), (nh_fm, 0, 512, 256),
                                     (nhh_fm, 512, 768, 512),
                                     (nh_fm, 256, 1280, 256)]
                        for mb in range(10):
                            band = w_hs_band(w["a1wk"], mb, NKT)
                            for (src, sc0, dc0, ncols) in kv_chunks:
                                ps = ps_mm.tile([128, 512], F32, tag="mm",
                                                name="mm")
                                for j in range(5):
                                    nc.tensor.matmul(
                                        ps[:, 0:ncols],
                                        band[:, 2 * j:2 * j + 2, :],
                                        src[:, 2 * j:2 * j + 2,
                                            sc0:sc0 + ncols],
                                        start=(j == 0), stop=(j == 4),
                                        perf_mode=DR)
                                dst = k_main[mb] if mb < 8 else k_rpk[mb - 8]
                                nc.any.tensor_copy(dst[:, dc0:dc0 + ncols],
                                                   ps[:, 0:ncols])

                        # V token-major over kv tokens: 12 tiles [128, 1280]
                        v_tm = [qkvp.tile([128, DIM], BF16, tag=f"v{i}",
                                          name=f"v{i}") for i in range(12)]
                        v_src = [(nhh_fm, 0), (nhh_fm, 128), (nhh_fm, 256),
                                 (nhh_fm, 384), (nh_fm, 0), (nh_fm, 128),
                                 (nhh_fm, 512), (nhh_fm, 640), (nhh_fm, 768),
                                 (nhh_fm, 896), (nh_fm, 256), (nh_fm, 384)]
                        for ch in range(4):
                            c0, c1 = 320 * ch, 320 * (ch + 1)
                            bnd = wmv.tile([128, NKT, 320], FP8,
                                           tag="wmv", name="wmv")
                            nc.sync.dma_start(bnd[:], w["a1wv"][ch])
                            for i, (src, sc0) in enumerate(v_src):
                                ps = ps_mm.tile([128, 512], F32, tag="mm",
                                                name="mm")
                                for j in range(5):
                                    nc.tensor.matmul(
                                        ps[:, 0:320],
                                        src[:, 2 * j:2 * j + 2,
                                            sc0:sc0 + 128],
                                        bnd[:, 2 * j:2 * j + 2, :],
                                        start=(j == 0), stop=(j == 4),
                                        perf_mode=DR)
                                nc.scalar.activation(v_tm[i][:, c0:c1],
                                                     ps[:, 0:320], AF.Copy,
                                                     scale=1.0 / WS)
                    # halo fm closed; Q projection (own tokens only)
                    q_main, q_rpk = project_headsplit(w["a1wq"], nh_fm, T_OWN,
                                                      qkvp, ps_mm, "q", NKT)

                # fm closed; attention core
                with tc.tile_pool(name="a1ao", bufs=1) as aop:
                    ao_fm = aop.tile([128, NKT, T_OWN], FP8, tag="aofm",
                                     name="aofm")
                    KB0 = [0, 3, 0, 3]
                    KB1 = [1, 4, 2, 5]
                    with tc.tile_pool(name="a1p", bufs=4) as pp:
                        for fi in range(4):
                            key_slices = [(256 * KB0[fi], 256, 0),
                                          (256 * KB1[fi], 256, 256)]
                            kvt = [2 * KB0[fi], 2 * KB0[fi] + 1,
                                   2 * KB1[fi], 2 * KB1[fi] + 1]
                            av_rem_ps = {}
                            for hd in range(HEADS):
                                g, j = hd // 4, hd % 4
                                pT = pp.tile([128, 4, 256], BF16, tag="pT",
                                             name="pT")
                                for qt in range(2):
                                    q0 = 256 * fi + 128 * qt
                                    sm = scores_psum(ps_mm, q_main, q_rpk,
                                                     k_main, k_rpk, hd,
                                                     slice(q0, q0 + 128),
                                                     key_slices)
                                    p = pp.tile([128, 512], BF16, tag="p", name="p")
                                    l = statp.tile([128, 1], F32, tag="l", name="l")
                                    nc.scalar.activation(p[:], sm[:], AF.Exp,
                                                         scale=ESC, accum_out=l[:])
                                    rinv = statp.tile([128, 1], F32, tag="rinv",
                                                      name="rinv")
                                    nc.vector.reciprocal(rinv[:], l[:])
                                    nc.vector.tensor_scalar_mul(p[:], p[:], rinv[:])
                                    tps = ps_tr.tile([128, 512], BF16, tag="tr", name="tr")
                                    for ki in range(4):
                                        nc.tensor.transpose(
                                            tps[:, 128 * ki:128 * (ki + 1)],
                                            p[:, 128 * ki:128 * (ki + 1)], ident[:])
                                        nc.any.tensor_copy(
                                            pT[:, ki, 128 * qt:128 * (qt + 1)],
                                            tps[:, 128 * ki:128 * (ki + 1)])
                                avp = ps_avm.tile([128, 256], F32, tag="avm",
                                                  name="avm")
                                for ki in range(4):
                                    nc.tensor.matmul(
                                        avp[:],
                                        v_tm[kvt[ki]][:, 160 * hd:160 * hd + 128],
                                        pT[:, ki, :],
                                        start=(ki == 0), stop=(ki == 3))
                                nc.any.tensor_copy(
                                    ao_fm[:, hd, 256 * fi:256 * (fi + 1)], avp[:])
                                if j == 0:
                                    av_rem_ps[g] = ps_avr.tile([128, 256], F32,
                                                               tag="avr", name="avr")
                                rps = av_rem_ps[g]
                                for ki in range(4):
                                    nc.tensor.matmul(
                                        rps[32 * j:32 * (j + 1), :],
                                        v_tm[kvt[ki]][:, 160 * hd + 128:160 * hd + 160],
                                        pT[:, ki, :],
                                        start=(ki == 0), stop=(ki == 3),
                                        tile_position=(0, 32 * j))
                                if j == 3:
                                    nc.any.tensor_copy(
                                        ao_fm[:, 8 + g, 256 * fi:256 * (fi + 1)],
                                        rps[:])

                    residual_project("a1bo", ao_fm, NKT, ps_mm, h, w["a1wo"])

            # =====================================================
            # Stage 2: attn2  (cross-attention to text)
            # =====================================================
            w_b = load_c("n2w", "lnw")
            b_b = load_c("n2b", "lnb")
            with tc.tile_pool(name="a2qkv", bufs=1) as qkvp, \
                 tc.tile_pool(name="a2ao", bufs=1) as aop, \
                 tc.tile_pool(name="ps_mm2", bufs=4, space="PSUM") as ps_mm, \
                 tc.tile_pool(name="ps_tr2", bufs=2, space="PSUM") as ps_tr, \
                 tc.tile_pool(name="ps_avm2", bufs=1, space="PSUM") as ps_avm, \
                 tc.tile_pool(name="ps_avr2", bufs=1, space="PSUM") as ps_avr:

                with tc.tile_pool(name="a2fm", bufs=1) as fmp:
                    with tc.tile_pool(name="lnscr2", bufs=3) as lnscr:
                        nh_tm = layernorm_rows(h, w_b, b_b, lnscr)
                        nh_fm = tm_to_fm(nh_tm, fmp, ps_tr, "nhfm", T_OWN, FP8)
                    q_main, q_rpk = project_headsplit(w["a2wq"], nh_fm, T_OWN,
                                                      qkvp, ps_mm, "q", NKT)

                ao_fm = aop.tile([128, NKT, T_OWN], FP8, tag="aofm",
                                 name="aofm")
                with tc.tile_pool(name="a2p", bufs=4) as pp:
                    for fi in range(4):
                        av_rem_ps = {}
                        for hd in range(HEADS):
                            g, j = hd // 4, hd % 4
                            pT = pp.tile([128, 256], BF16, tag="pT", name="pT")
                            for qt in range(2):
                                q0 = 256 * fi + 128 * qt
                                sm = scores_psum(
                                    ps_mm, q_main, q_rpk, k2_main, k2_rpk,
                                    hd, slice(q0, q0 + 128),
                                    [(77 * fi, 77, 0)])
                                p = pp.tile([128, 128], BF16, tag="p", name="p")
                                l = statp.tile([128, 1], F32, tag="l", name="l")
                                nc.scalar.activation(p[:, 0:77], sm[:, 0:77],
                                                     AF.Exp, scale=ESC,
                                                     accum_out=l[:])
                                rinv = statp.tile([128, 1], F32, tag="rinv",
                                                  name="rinv")
                                nc.vector.reciprocal(rinv[:], l[:])
                                nc.vector.tensor_scalar_mul(p[:, 0:77],
                                                            p[:, 0:77], rinv[:])
                                tps = ps_tr.tile([128, 128], BF16, tag="tr",
                                                 name="tr")
                                nc.tensor.transpose(tps[0:77, :], p[:, 0:77],
                                                    ident[:])
                                nc.any.tensor_copy(
                                    pT[0:77, 128 * qt:128 * (qt + 1)],
                                    tps[0:77, :])
                            avp = ps_avm.tile([128, 256], F32, tag="avm",
                                              name="avm")
                            nc.tensor.matmul(avp[:],
                                             v2[fi][0:77, 160 * hd:160 * hd + 128],
                                             pT[0:77, :], start=True, stop=True)
                            nc.any.tensor_copy(
                                ao_fm[:, hd, 256 * fi:256 * (fi + 1)], avp[:])
                            if j == 0:
                                av_rem_ps[g] = ps_avr.tile([128, 256], F32,
                                                           tag="avr", name="avr")
                            rps = av_rem_ps[g]
                            nc.tensor.matmul(
                                rps[32 * j:32 * (j + 1), :],
                                v2[fi][0:77, 160 * hd + 128:160 * hd + 160],
                                pT[0:77, :], start=True, stop=True,
                                tile_position=(0, 32 * j))
                            if j == 3:
                                nc.any.tensor_copy(
                                    ao_fm[:, 8 + g, 256 * fi:256 * (fi + 1)],
                                    rps[:])

                residual_project("a2bo", ao_fm, NKT, ps_mm, h, w["a2wo"])
            a2stack.close()

            # =====================================================
            # Stage 3: geglu FFN (bf16) + interleaved reshard AllToAlls
            # =====================================================
            w_b = load_c("n3w", "lnw")
            b_b = load_c("n3b", "lnb")
            cins = [dramp.tile([8, 64, DIM], BF16, tag=f"cin{hh}",
                               name=f"cin{hh}") for hh in range(2)]
            couts = dramp.tile([2, 8, 64, DIM], BF16, tag="couts",
                               name="couts")
            with tc.tile_pool(name="ffp", bufs=1) as ffp, \
                 tc.tile_pool(name="gelu", bufs=1) as gelup:

                ff_all = ffp.tile([128, NM1, T_OWN], BF16, tag="ff",
                                  name="ff")
                with tc.tile_pool(name="f3fm", bufs=1) as fmp:
                    with tc.tile_pool(name="lnscr3", bufs=3) as lnscr, \
                         tc.tile_pool(name="ps_tr3", bufs=2,
                                      space="PSUM") as ps_tr3:
                        nh_tm = layernorm_rows(h, w_b, b_b, lnscr)
                        nh_fm = tm_to_fm(nh_tm, fmp, ps_tr3, "nhfm", T_OWN,
                                         BF16)

                    # W1: one band load serves both token chunks and p/g
                    with tc.tile_pool(name="ps_pg", bufs=4,
                                      space="PSUM") as ps_pg, \
                         tc.tile_pool(name="w1bp", bufs=2) as w1bp:
                        bb = load_c("ffb2", "obias")
                        for t in range(NT_OWN):
                            nc.vector.tensor_add(h[t][:], h[t][:], bb[:])

                        for m in range(NM1):
                            w1b = w1bp.tile([128, NKT, 2, 128], BF16,
                                            tag="w1b", name="w1b")
                            nc.sync.dma_start(w1b[:], w["ffw1"][m])
                            pgs = [ps_pg.tile([128, 512], F32, tag="pg",
                                              name="pg") for _ in range(4)]
                            for kt in range(NKT):
                                for pg in range(2):
                                    for tci in range(2):
                                        nc.tensor.matmul(
                                            pgs[2 * pg + tci][:],
                                            w1b[:, kt, pg, :],
                                            nh_fm[:, kt,
                                                  512 * tci:512 * (tci + 1)],
                                            start=(kt == 0),
                                            stop=(kt == NKT - 1))
                            for tci in range(2):
                                tc0 = 512 * tci
                                gp = gelup.tile([128, 512], BF16, tag="gp",
                                                name="gp")
                                nc.scalar.activation(gp[:], pgs[2 + tci][:],
                                                     AF.Gelu,
                                                     bias=b1g_sb[:, m:m + 1])
                                nc.vector.scalar_tensor_tensor(
                                    ff_all[:, m, tc0:tc0 + 512], pgs[tci][:],
                                    b1p_sb[:, m:m + 1], gp[:],
                                    ALU.add, ALU.mult)

                # W2 in two halves; AllToAll of each half right after it
                # completes so collective overlaps remaining compute
                with tc.tile_pool(name="ps_w2", bufs=4, space="PSUM") as ps_w2, \
                     tc.tile_pool(name="w2bp", bufs=2) as w2bp, \
                     tc.tile_pool(name="cstg", bufs=1) as cstg:
                    for hh in range(2):
                        for ch in range(4):
                            c0, c1 = 320 * ch, 320 * (ch + 1)
                            w2b = [w2bp.tile([128, 20, 320], BF16,
                                             tag=f"w2b{mh}", name=f"w2b{mh}")
                                   for mh in range(2)]
                            for mh in range(2):
                                nc.sync.dma_start(w2b[mh][:], w["ffw2"][ch, mh])
                            for tt in range(4 * hh, 4 * hh + 4):
                                ps = ps_w2.tile([128, 512], F32, tag="w2",
                                                name="w2")
                                for m in range(NM1):
                                    nc.tensor.matmul(
                                        ps[:, 0:320],
                                        ff_all[:, m, 128 * tt:128 * (tt + 1)],
                                        w2b[m // 20][:, m % 20, :],
                                        start=(m == 0), stop=(m == NM1 - 1))
                                nc.vector.tensor_add(h[tt][:, c0:c1],
                                                     h[tt][:, c0:c1],
                                                     ps[:, 0:320])
                        # this half of h is final: stage + send its AllToAll
                        csts = []
                        for q in range(4):
                            cst = cstg.tile([128, DIM], BF16, tag=f"cs{q}",
                                            name=f"cs{q}")
                            nc.any.tensor_copy(cst[:], h[4 * hh + q][:])
                            csts.append(cst)
                        for jj in range(8):
                            u = 2 * hh + jj // 4
                            r0 = 256 * u + 64 * (jj % 4)
                            lt = r0 // 128 - 4 * hh
                            ro = r0 % 128
                            nc.sync.dma_start(cins[hh][jj],
                                              csts[lt][ro:ro + 64, :])
                        nc.gpsimd.collective_compute(
                            "AllToAll", ALU.bypass,
                            replica_groups=[[0, 1, 2, 3, 4, 5, 6, 7]],
                            ins=[cins[hh].opt()], outs=[couts[hh].opt()])

            # reload resharded rows: row = dl*16 + hh*8 + pc
            # (frame = 2*pc + hh; host unpermutes)
            h4stack = contextlib.ExitStack()
            h4pool = h4stack.enter_context(tc.tile_pool(name="h4", bufs=1))
            h4 = []
            for i in range(8):
                t4 = h4pool.tile([128, DIM], BF16, tag=f"h4_{i}",
                                 name=f"h4_{i}")
                nc.sync.dma_start(
                    t4[:],
                    couts[:, :, 8 * i:8 * (i + 1), :]
                    .rearrange("h p d c -> d h p c"))
                h4.append(t4)

            # =====================================================
            # Stage 4: temporal self-attention over frames
            # =====================================================
            w_b = load_c("ntw", "lnw")
            b_b = load_c("ntb", "lnb")
            with tc.tile_pool(name="tqkv", bufs=1) as qkvp, \
                 tc.tile_pool(name="tao", bufs=1) as aop, \
                 tc.tile_pool(name="ps_mmt", bufs=4, space="PSUM") as ps_mm, \
                 tc.tile_pool(name="ps_trt", bufs=2, space="PSUM") as ps_tr, \
                 tc.tile_pool(name="ps_avmt", bufs=1, space="PSUM") as ps_avm, \
                 tc.tile_pool(name="ps_avrt", bufs=1, space="PSUM") as ps_avr:

                with tc.tile_pool(name="tfm", bufs=1) as fmp:
                    with tc.tile_pool(name="lnscrt", bufs=3) as lnscr:
                        nh_tm = layernorm_rows(h4, w_b, b_b, lnscr)
                        nh_fm = tm_to_fm(nh_tm, fmp, ps_tr, "nhfm", T_OWN, FP8)

                    q_main, q_rpk = project_headsplit(w["atwq"], nh_fm, T_OWN,
                                                      qkvp, ps_mm, "q", NKT)
                    k_main, k_rpk = project_headsplit(w["atwk"], nh_fm, T_OWN,
                                                      qkvp, ps_mm, "k", NKT)
                    v_tm = [qkvp.tile([128, DIM], BF16, tag=f"v{i}",
                                      name=f"v{i}") for i in range(8)]
                    for ch in range(4):
                        c0, c1 = 320 * ch, 320 * (ch + 1)
                        bnd = wmv.tile([128, NKT, 320], FP8, tag="wmv",
                                       name="wmv")
                        nc.sync.dma_start(bnd[:], w["atwv"][ch])
                        for t in range(8):
                            ps = ps_mm.tile([128, 512], F32, tag="mm",
                                            name="mm")
                            for j in range(5):
                                nc.tensor.matmul(
                                    ps[:, 0:320],
                                    nh_fm[:, 2 * j:2 * j + 2,
                                          128 * t:128 * (t + 1)],
                                    bnd[:, 2 * j:2 * j + 2, :],
                                    start=(j == 0), stop=(j == 4),
                                    perf_mode=DR)
                            nc.scalar.activation(v_tm[t][:, c0:c1],
                                                 ps[:, 0:320], AF.Copy,
                                                 scale=1.0 / WS)

                ao_fm = aop.tile([128, NKT, T_OWN], FP8, tag="aofm",
                                 name="aofm")
                with tc.tile_pool(name="tp", bufs=4) as pp:
                    for gdx in range(8):
                        g0 = 128 * gdx
                        av_rem_ps = {}
                        for hd in range(HEADS):
                            g, j = hd // 4, hd % 4
                            sm = scores_psum(ps_mm, q_main, q_rpk,
                                             k_main, k_rpk, hd,
                                             slice(g0, g0 + 128),
                                             [(g0, 128, 0)])
                            p = pp.tile([128, 128], BF16, tag="p", name="p")
                            nc.scalar.activation(p[:], sm[:, 0:128], AF.Exp,
                                                 scale=ESC)
                            l = statp.tile([128, 1], F32, tag="l", name="l")
                            nc.vector.scalar_tensor_tensor(p[:], p[:], 1.0,
                                                           mask_sb[:], ALU.mult,
                                                           ALU.mult,
                                                           accum_out=l[:])
                            rinv = statp.tile([128, 1], F32, tag="rinv",
                                              name="rinv")
                            nc.vector.reciprocal(rinv[:], l[:])
                            nc.vector.tensor_scalar_mul(p[:], p[:], rinv[:])
                            tps = ps_tr.tile([128, 128], BF16, tag="tr",
                                             name="tr")
                            nc.tensor.transpose(tps[:], p[:], ident[:])
                            pT = pp.tile([128, 128], BF16, tag="pT", name="pT")
                            nc.any.tensor_copy(pT[:], tps[:])
                            avp = ps_avm.tile([128, 128], F32, tag="avm",
                                              name="avm")
                            nc.tensor.matmul(avp[:],
                                             v_tm[gdx][:, 160 * hd:160 * hd + 128],
                                             pT[:], start=True, stop=True)
                            nc.any.tensor_copy(ao_fm[:, hd, g0:g0 + 128],
                                               avp[:])
                            if j == 0:
                                av_rem_ps[g] = ps_avr.tile([128, 128], F32,
                                                           tag="avr", name="avr")
                            rps = av_rem_ps[g]
                            nc.tensor.matmul(
                                rps[32 * j:32 * (j + 1), :],
                                v_tm[gdx][:, 160 * hd + 128:160 * hd + 160],
                                pT[:], start=True, stop=True,
                                tile_position=(0, 32 * j))
                            if j == 3:
                                nc.any.tensor_copy(ao_fm[:, 8 + g, g0:g0 + 128],
                                                   rps[:])

                residual_project("atbo", ao_fm, NKT, ps_mm, h4, w["atwo"])

            for t in range(NT_OWN):
                nc.sync.dma_start(out_d[128 * t:128 * (t + 1), :], h4[t][:])
            h4stack.close()

    nc.compile()
    return nc


# ================= host side =================

def _prep_inputs(inputs):
    hs = np.ascontiguousarray(np.asarray(inputs["hidden_states"], np.float32))
    enc = np.ascontiguousarray(np.asarray(inputs["encoder_hidden_states"],
                                          np.float32))
    vl = int(np.asarray(inputs["video_length"]))
    assert vl == FRAMES and hs.shape == (B * FRAMES, TOK, DIM)

    def _q8(x):
        return np.clip(x, -240.0, 240.0).astype(fp8e4)

    def _hs_tiles(wt):
        """[Kin, 1280] -> [10 mb, 128 p, nkt, 128 c] head-split bands."""
        kin = wt.shape[0]
        nkt = kin // 128
        out = np.empty((10, 128, nkt, 128), np.float32)
        w3 = wt.reshape(nkt, 128, HEADS, DH)   # [kt, p, h, c]
        for mb in range(8):
            out[mb] = w3[:, :, mb, 0:128].transpose(1, 0, 2)
        for g in range(2):
            rem = w3[:, :, 4 * g:4 * (g + 1), 128:160]  # [kt, p, 4, 32]
            out[8 + g] = rem.reshape(nkt, 128, 128).transpose(1, 0, 2)
        return out

    def _mv_tiles(wt):
        """[Kin, 1280] -> [4 ch, 128 p, nkt, 320] moving bands."""
        kin = wt.shape[0]
        nkt = kin // 128
        return np.ascontiguousarray(
            wt.reshape(nkt, 128, 4, 320).transpose(2, 1, 0, 3))

    def _wo_perm(wt):
        """Permute O-proj rows into head-split order, then moving bands."""
        w3 = wt.reshape(HEADS, DH, DIM)
        rows = [w3[hd, 0:128] for hd in range(8)]
        rows += [w3[4 * g:4 * (g + 1), 128:160].reshape(128, DIM)
                 for g in range(2)]
        return _mv_tiles(np.concatenate(rows, 0))

    gw = lambda k: np.asarray(inputs[k], np.float32)
    ffw1 = gw("ffw1")
    ffw1_t = np.empty((NM1, 128, NKT, 2, 128), np.float32)
    for m in range(NM1):
        for kt in range(NKT):
            ks = slice(128 * kt, 128 * (kt + 1))
            ffw1_t[m, :, kt, 0, :] = ffw1[ks, 128 * m:128 * (m + 1)]
            ffw1_t[m, :, kt, 1, :] = ffw1[ks,
                                          INNER + 128 * m:INNER + 128 * (m + 1)]
    # W2 [5120, 1280] -> [4 ch, 2 mh, 128 p, 20 m2, 320]
    ffw2_t = np.ascontiguousarray(
        gw("ffw2").reshape(2, 20, 128, 4, 320).transpose(3, 0, 2, 1, 4))

    # fp8 weights, prescaled x WS
    wb8 = {
        "a1wq": _hs_tiles(gw("a1wq")), "a1wk": _hs_tiles(gw("a1wk")),
        "a2wq": _hs_tiles(gw("a2wq")), "a2wk": _hs_tiles(gw("a2wk")),
        "atwq": _hs_tiles(gw("atwq")), "atwk": _hs_tiles(gw("atwk")),
        "a1wv": _mv_tiles(gw("a1wv")), "a2wv": _mv_tiles(gw("a2wv")),
        "atwv": _mv_tiles(gw("atwv")),
        "a1wo": _wo_perm(gw("a1wo")), "a2wo": _wo_perm(gw("a2wo")),
        "atwo": _wo_perm(gw("atwo")),
    }
    wb8 = {k: np.ascontiguousarray(_q8(v * WS)) for k, v in wb8.items()}
    wb = {"ffw1": np.ascontiguousarray(ffw1_t.astype(bf16)),
          "ffw2": np.ascontiguousarray(ffw2_t.astype(bf16))}
    bc = {}
    for k in ["n1w", "n1b", "n2w", "n2b", "n3w", "n3b", "ntw", "ntb",
              "a1bo", "a2bo", "ffb2", "atbo"]:
        v = np.asarray(inputs[k], np.float32)
        bc[k + "_bc"] = np.ascontiguousarray(
            np.broadcast_to(v[None, :], (128, DIM)).astype(bf16))
    ffb1 = np.asarray(inputs["ffb1"], np.float32)
    ffb1p = np.ascontiguousarray(ffb1[:INNER].reshape(NM1, 128).T)
    ffb1g = np.ascontiguousarray(ffb1[INNER:].reshape(NM1, 128).T)
    tmask = np.ascontiguousarray(
        np.kron(np.eye(8, dtype=np.float32),
                np.ones((16, 16), np.float32)).astype(bf16))

    in_maps = []
    for c in range(N_CORES):
        f0 = 2 * c
        fp = max(f0 - 1, 0)
        units = [(0, f0), (1, f0), (0, f0 + 1), (1, f0 + 1)]
        h_own = np.concatenate([hs[b * FRAMES + f] for (b, f) in units], 0)
        h_halo = np.concatenate([hs[0], hs[fp], hs[FRAMES],
                                 hs[FRAMES + fp]], 0).astype(bf16)
        enc_c = np.concatenate([enc[b * FRAMES + f] for (b, f) in units], 0)
        # feature-major fp8, padded to TENC cols: [128, NKTC, TENC]
        enc_fm = np.zeros((128, NKTC, TENC), np.float32)
        ka = enc_c.T.reshape(NKTC, 128, 4 * ESEQ).transpose(1, 0, 2)
        enc_fm[:, :, :4 * ESEQ] = ka
        m = {"h_own": np.ascontiguousarray(h_own),
             "h_halo": np.ascontiguousarray(h_halo),
             "enc_fm": np.ascontiguousarray(_q8(enc_fm)),
             "ffb1p": ffb1p, "ffb1g": ffb1g, "tmask": tmask}
        m.update(wb8)
        m.update(wb)
        m.update(bc)
        in_maps.append(m)
    return in_maps


# row position of frame f within a 16-row dl group after the reshard
_POSOF = [(f % 2) * 8 + f // 2 for f in range(FRAMES)]


def _assemble(results):
    full = np.empty((B, FRAMES, TOK, DIM), np.float32)
    for c in range(N_CORES):
        o = np.asarray(results[c]["out"], np.float32).reshape(64, FRAMES, DIM)
        b, d0 = c // 4, 64 * (c % 4)
        full[b, :, d0:d0 + 64, :] = o[:, _POSOF, :].transpose(1, 0, 2)
    return full.reshape(B * FRAMES, TOK, DIM)


def _get_nc():
    if "nc" not in _CACHE:
        _CACHE["nc"] = build_program()
    return _CACHE["nc"]


def kernel(**inputs):
    nc = _get_nc()
    in_maps = _prep_inputs(inputs)
    res = bass_utils.run_bass_kernel_spmd(nc, in_maps,
                                          core_ids=list(range(N_CORES)))
    return _assemble(res.results)
